# revision 32
# baseline (speedup 1.0000x reference)
"""GQA attention block on 8 trn2 NeuronCores.

Sharding: core c = (batch b=c//4, kv-head-pair g=c%4). Each core owns kv heads
{2g, 2g+1} and their 8 query heads (GQA tile mapping: q-head i -> kv-head i%8),
with Wq/Wk/Wv column-sharded and Wo row-sharded; host sums the 4 partial
outputs per batch (bf16 partials, fp32 sum) and adds bo.

Device strategy (per core):
  - host stages q^T/k^T/v^T (bf16) so every matmul has its contraction dim on
    partitions with no device-side transposes.
  - RoPE applied on DVE (partition-shifted rotate_half + cos/sin combine) for
    both Q and K; no doubled projection weights.
  - q heads are interleaved as (kv0-head j, kv1-head j) pairs so each score
    matmul pair runs ROW-TILED on the PE array (64x128 tiles T0/T8, concurrent)
    writing a 2-bank PSUM blob; one Exp ACT over the [128,1024] blob (scale=1/8
    folded, no max subtraction -- scores bounded ~|6|).
  - AV via lhsT=Vp with an appended ones column giving the softmax denominator
    for free; normalization via fast-approx reciprocal + partition broadcast.
  - out^T feeds the final projection as lhsT directly; partial [S,D] bf16 out.
"""

import os
from contextlib import ExitStack

import numpy as np
import ml_dtypes

D = 2048
QH = 32
KVH = 8
HD = 64
B = 2
S = 2048
THETA = 1000000.0
P = 128
NCORES = 8

BF16 = ml_dtypes.bfloat16

_CACHE = {}


def _build_program():
    import concourse.bass as bass
    import concourse.tile as tile
    from concourse import bacc, mybir

    nc = bacc.Bacc(
        "TRN2",
        target_bir_lowering=False,
        debug=False,
        enable_asserts=False,
        num_devices=NCORES,
    )
    bf = mybir.dt.bfloat16
    f32 = mybir.dt.float32
    Exp = mybir.ActivationFunctionType.Exp
    scale = 1.0 / float(np.sqrt(HD))

    qT = nc.dram_tensor("qT", [D, S], bf, kind="ExternalInput").ap()
    kT = nc.dram_tensor("kT", [D, S], bf, kind="ExternalInput").ap()
    vT = nc.dram_tensor("vT", [D, S], bf, kind="ExternalInput").ap()
    wq = nc.dram_tensor("wq", [D, 512], bf, kind="ExternalInput").ap()
    wk = nc.dram_tensor("wk", [D, 128], bf, kind="ExternalInput").ap()
    wv = nc.dram_tensor("wv", [D, 128], bf, kind="ExternalInput").ap()
    wo = nc.dram_tensor("wo", [512, D], bf, kind="ExternalInput").ap()
    cosr = nc.dram_tensor("cosr", [P, S], bf, kind="ExternalInput").ap()
    sinr = nc.dram_tensor("sinr", [P, S], bf, kind="ExternalInput").ap()
    ident = nc.dram_tensor("ident", [P, P], bf, kind="ExternalInput").ap()
    out = nc.dram_tensor("out", [S, D], bf, kind="ExternalOutput").ap()

    # partitioned DRAM views
    qT3 = qT.rearrange("(o p) s -> p o s", p=P)    # [128, 16, 2048]
    kT3 = kT.rearrange("(o p) s -> p o s", p=P)
    vT3 = vT.rearrange("(o p) s -> p o s", p=P)
    wq3 = wq.rearrange("(o p) m -> p o m", p=P)    # [128, 16, 512]
    wk3 = wk.rearrange("(o p) m -> p o m", p=P)    # [128, 16, 128]
    wv3 = wv.rearrange("(o p) m -> p o m", p=P)    # [128, 16, 128]
    wo3 = wo.rearrange("(o p) d -> p o d", p=P)    # [128, 4, 2048]
    out3 = out.rearrange("(t p) d -> p t d", p=P)  # [128, 16, 2048]

    with tile.TileContext(nc) as tc, ExitStack() as ctx:
        const = ctx.enter_context(tc.tile_pool(name="const", bufs=1))
        persist = ctx.enter_context(tc.tile_pool(name="persist", bufs=1))

        # ---- resident weights / tables (DMA order = need order; wo is
        # deferred until after startup) ----
        wq_sb = const.tile([P, 16, 512], bf, tag="wq")
        nc.sync.dma_start(wq_sb[:], wq3[:])
        wv_sb = const.tile([P, 16, 128], bf, tag="wv")
        nc.sync.dma_start(wv_sb[:], wv3[:])
        wk_sb = const.tile([P, 16, 128], bf, tag="wk")
        nc.sync.dma_start(wk_sb[:], wk3[:])
        cos_sb = const.tile([P, S], bf, tag="cos")
        nc.sync.dma_start(cos_sb[:], cosr[:])
        sin_sb = const.tile([P, S], bf, tag="sin")
        nc.sync.dma_start(sin_sb[:], sinr[:])
        ident_sb = const.tile([P, P], bf, tag="ident")
        nc.sync.dma_start(ident_sb[:], ident[:])
        wo_sb = const.tile([P, 4, 2048], bf, tag="wo")
        # full q^T resident (one-time 4KB-run loads; quarters consumed lazily)
        qfull = persist.tile([P, 16, 2048], bf, tag="qfull")

        # ---- persistent intermediates ----
        kpt_b = persist.tile([P, S], bf, tag="kpt")          # rotated K^T, kv0|kv1
        qpt_b = persist.tile([P, 4, S], bf, tag="qpt")       # rotated Q^T pairs
        vp_sb = persist.tile([P, 16, 130], bf, tag="vp")     # Vp + ones cols
        outT_b = persist.tile([P, 4, S], bf, tag="outT")     # normalized out^T
        nc.vector.memset(vp_sb[:, :, 64:65], 1.0)
        nc.vector.memset(vp_sb[:, :, 129:130], 1.0)

        def rope_combine(dst, ps, rot_pool, sl):
            """dst[128, 512] (bf16) = ps*cos + rotate_half(ps)*sin over slice sl."""
            rot = rot_pool.tile([P, 512], f32, tag="rot")
            for hh in range(2):
                b0 = hh * 64
                nc.vector.tensor_scalar_mul(
                    rot[b0 : b0 + 32, :], ps[b0 + 32 : b0 + 64, :], -1.0
                )
                nc.vector.tensor_copy(
                    out=rot[b0 + 32 : b0 + 64, :], in_=ps[b0 : b0 + 32, :]
                )
            t1 = rot_pool.tile([P, 512], f32, tag="t1")
            t2 = rot_pool.tile([P, 512], f32, tag="t2")
            nc.vector.tensor_mul(out=t1[:], in0=ps[:], in1=cos_sb[:, sl])
            nc.vector.tensor_mul(out=t2[:], in0=rot[:], in1=sin_sb[:, sl])
            nc.vector.tensor_add(out=dst, in0=t1[:], in1=t2[:])

        # ======= per s-quarter: attention with interleaved Q/O proj =======
        with ExitStack() as mctx:
            ptmp = mctx.enter_context(tc.tile_pool(name="ptmp2", bufs=1))
            epool = mctx.enter_context(tc.tile_pool(name="et", bufs=3))
            ntmp = mctx.enter_context(tc.tile_pool(name="ntmp", bufs=1))
            fout = mctx.enter_context(tc.tile_pool(name="fout", bufs=1))

            def qproj_steps(quarter, pools):
                """64 tensor-step closures; each emits one matmul (rope attached
                to the last o of each m)."""
                gs = slice(quarter * 512, (quarter + 1) * 512)
                state = {}

                def step(m, o):
                    if o == 0:
                        pool, tag = pools[m % len(pools)]
                        state["ps"] = pool.tile(
                            [P, 512], f32, tag=tag, name=f"qp{quarter}_{m}"
                        )
                    nc.tensor.matmul(
                        state["ps"],
                        lhsT=wq_sb[:, o, m * 128 : (m + 1) * 128],
                        rhs=qfull[:, o, gs],
                        start=(o == 0),
                        stop=(o == 15),
                    )
                    if o == 15:
                        rope_combine(qpt_b[:, m, gs], state["ps"], ptmp, gs)

                return [
                    (lambda m=m, o=o: step(m, o))
                    for m in range(4)
                    for o in range(16)
                ]

            def oproj_steps(quarter, pools):
                """64 tensor-step closures; the 4 dn results of each s-tile are
                staged in one [P,2048] bf16 tile and written with a single
                4KB-run DMA."""
                state = {}

                def step(i, qt, dn, cc):
                    if cc == 0:
                        pool, tag = pools[i % len(pools)]
                        state["psf"] = pool.tile(
                            [P, 512], f32, tag=tag, name=f"psf{quarter}_{i}"
                        )
                        if dn == 0:
                            state["of"] = fout.tile(
                                [P, 2048], bf, tag="of", name=f"of{quarter}_{qt}"
                            )
                    nc.tensor.matmul(
                        state["psf"],
                        lhsT=outT_b[:, cc, qt * 128 : (qt + 1) * 128],
                        rhs=wo_sb[:, cc, dn * 512 : (dn + 1) * 512],
                        start=(cc == 0),
                        stop=(cc == 3),
                    )
                    if cc == 3:
                        nc.vector.tensor_copy(
                            out=state["of"][:, dn * 512 : (dn + 1) * 512],
                            in_=state["psf"][:],
                        )
                        if dn == 3:
                            nc.sync.dma_start(out3[:, qt, :], state["of"][:])

                return [
                    (
                        lambda i=qi * 4 + dn, qt=quarter * 4 + qi, dn=dn, cc=cc:
                        step(i, qt, dn, cc)
                    )
                    for qi in range(4)
                    for dn in range(4)
                    for cc in range(4)
                ]

            def attention(quarter, feeds):
                gs = slice(quarter * 512, (quarter + 1) * 512)
                fed = 0
                nslots = 64
                def emit_av(avA, avB, et, c):
                    nc.tensor.matmul(
                        avA[:],
                        lhsT=vp_sb[:, c, 0:65],
                        rhs=et[:, 0:512],
                        start=(c == 0),
                        stop=(c == 15),
                    )
                    nc.tensor.matmul(
                        avB[:],
                        lhsT=vp_sb[:, c, 65:130],
                        rhs=et[:, 512:1024],
                        start=(c == 0),
                        stop=(c == 15),
                    )

                for j in range(4):
                    avA = avpsum.tile([65, 512], f32, tag="avA")
                    avB = avpsum.tile([65, 512], f32, tag="avB")
                    pend = None  # (et, c) whose AV is deferred one slot
                    for c in range(16):
                        cs = slice(c * 128, (c + 1) * 128)
                        sb = spsum.tile([P, 1024], f32, tag="sb")
                        nc.tensor.matmul(
                            sb[:, 0:512],
                            lhsT=kpt_b[0:64, cs],
                            rhs=qpt_b[0:64, j, gs],
                            start=True,
                            stop=True,
                            tile_position=(0, 0),
                        )
                        nc.tensor.matmul(
                            sb[:, 512:1024],
                            lhsT=kpt_b[64:128, cs],
                            rhs=qpt_b[64:128, j, gs],
                            start=True,
                            stop=True,
                            tile_position=(64, 0),
                        )
                        et = epool.tile([P, 1024], bf, tag="et", name=f"et{c}")
                        nc.scalar.activation(
                            out=et[:], in_=sb[:], func=Exp, scale=scale
                        )
                        # feeds run while this chunk's exp is on the ACT engine
                        slot = j * 16 + c
                        want = (slot + 1) * len(feeds) // nslots
                        while fed < want:
                            feeds[fed]()
                            fed += 1
                        if pend is not None:
                            emit_av(avA, avB, *pend)
                        pend = (et, c)
                    emit_av(avA, avB, *pend)
                    # stage AV out of PSUM immediately (frees the banks for the
                    # next pair); normalize from SBUF afterwards. den rows land
                    # at partition 0 (required by the custom-DVE reciprocal).
                    sdimA = ntmp.tile([64, 512], bf, tag="sdimA")
                    denA = ntmp.tile([1, 512], f32, tag="denA")
                    sdimB = ntmp.tile([64, 512], bf, tag="sdimB")
                    denB = ntmp.tile([1, 512], f32, tag="denB")
                    nc.vector.tensor_copy(out=sdimA[:], in_=avA[0:64, :])
                    nc.vector.tensor_copy(out=denA[:], in_=avA[64:65, :])
                    nc.vector.tensor_copy(out=sdimB[:], in_=avB[0:64, :])
                    nc.vector.tensor_copy(out=denB[:], in_=avB[64:65, :])
                    recA = ntmp.tile([1, 512], f32, tag="recA")
                    recB = ntmp.tile([1, 512], f32, tag="recB")
                    nc.vector.reciprocal_approx_fast(out=recA[:], in_=denA[:])
                    nc.vector.reciprocal_approx_fast(out=recB[:], in_=denB[:])
                    bcA = ntmp.tile([64, 512], f32, tag="bcA")
                    bcB = ntmp.tile([64, 512], f32, tag="bcB")
                    nc.gpsimd.partition_broadcast(bcA[:], recA[:])
                    nc.gpsimd.partition_broadcast(bcB[:], recB[:])
                    nc.vector.tensor_mul(
                        out=outT_b[0:64, j, gs], in0=sdimA[:], in1=bcA[:]
                    )
                    nc.vector.tensor_mul(
                        out=outT_b[64:128, j, gs], in0=sdimB[:], in1=bcB[:]
                    )
                while fed < len(feeds):
                    feeds[fed]()
                    fed += 1

            def interleave(a, b):
                out = []
                for x, y in zip(a, b):
                    out.append(x)
                    out.append(y)
                out += a[len(b):] or b[len(a):]
                return out

            # ===== startup =====
            # qfull loads first (4KB runs); then qproj(0) standalone with its
            # own 2-bank PSUM scope; then V (VpT + PE transpose), then K
            # (4 ns-block accumulators) -- all full-row 4KB-run streams.
            for o in range(16):
                nc.sync.dma_start(qfull[:, o, :], qT3[:, o, :])

            with ExitStack() as q0ctx:
                q0psum = q0ctx.enter_context(
                    tc.tile_pool(name="q0psum", bufs=2, space="PSUM")
                )
                for s in qproj_steps(0, [(q0psum, "qp0")]):
                    s()

            with ExitStack() as vctx:
                instream = vctx.enter_context(
                    tc.tile_pool(name="instream", bufs=4)
                )
                vpsum = vctx.enter_context(
                    tc.tile_pool(name="vpsum", bufs=4, space="PSUM")
                )
                trpool = vctx.enter_context(
                    tc.tile_pool(name="trpool", bufs=2, space="PSUM")
                )
                psvT = [
                    vpsum.tile([P, 512], f32, tag="pv", name=f"pv{q}")
                    for q in range(4)
                ]
                for o in range(16):
                    vt = instream.tile([P, 2048], bf, tag="ins", name=f"v{o}")
                    nc.sync.dma_start(vt[:], vT3[:, o, :])
                    for q in range(4):
                        nc.tensor.matmul(
                            psvT[q],
                            lhsT=wv_sb[:, o, :],
                            rhs=vt[:, q * 512 : (q + 1) * 512],
                            start=(o == 0),
                            stop=(o == 15),
                        )
                for q in range(4):
                    vts = ptmp.tile([P, 512], bf, tag="vts", name=f"vts{q}")
                    nc.vector.tensor_copy(out=vts[:], in_=psvT[q])
                    for st in range(4):
                        tr = trpool.tile([P, P], bf, tag="tr", name=f"tr{q}_{st}")
                        nc.tensor.transpose(
                            tr[:], vts[:, st * 128 : (st + 1) * 128], ident_sb[:]
                        )
                        c = q * 4 + st
                        nc.vector.tensor_copy(
                            out=vp_sb[:, c, 0:64], in_=tr[:, 0:64]
                        )
                        nc.vector.tensor_copy(
                            out=vp_sb[:, c, 65:129], in_=tr[:, 64:128]
                        )

            with ExitStack() as kctx:
                instream = kctx.enter_context(
                    tc.tile_pool(name="instream2", bufs=4)
                )
                kpsum = kctx.enter_context(
                    tc.tile_pool(name="kpsum", bufs=4, space="PSUM")
                )
                ps_k = [
                    kpsum.tile([P, 512], f32, tag="pk", name=f"pk{ns}")
                    for ns in range(4)
                ]
                for o in range(16):
                    kt_t = instream.tile([P, 2048], bf, tag="ins", name=f"k{o}")
                    nc.sync.dma_start(kt_t[:], kT3[:, o, :])
                    for ns in range(4):
                        nc.tensor.matmul(
                            ps_k[ns],
                            lhsT=wk_sb[:, o, :],
                            rhs=kt_t[:, ns * 512 : (ns + 1) * 512],
                            start=(o == 0),
                            stop=(o == 15),
                        )
                for ns in range(4):
                    sl = slice(ns * 512, (ns + 1) * 512)
                    rope_combine(kpt_b[:, sl], ps_k[ns], ptmp, sl)

            # attention-phase PSUM pools (startup pools have been released)
            qpool = mctx.enter_context(
                tc.tile_pool(name="qpool", bufs=1, space="PSUM")
            )
            opool = mctx.enter_context(
                tc.tile_pool(name="opool", bufs=1, space="PSUM")
            )
            spsum = mctx.enter_context(
                tc.tile_pool(name="spsum", bufs=2, space="PSUM")
            )
            avpsum = mctx.enter_context(
                tc.tile_pool(name="avpsum", bufs=1, space="PSUM")
            )
            nc.sync.dma_start(wo_sb[:], wo3[:])

            QP = (qpool, "qp")
            OP = (opool, "psf")

            # Balanced feeds: 96 proj steps per quarter (1.5/slot) -- Q
            # projections split across quarter boundaries so the tensor
            # engine stays just under the exp-ACT pace everywhere.
            q1s = qproj_steps(1, [QP])
            q2s = qproj_steps(2, [QP])
            attention(0, q1s + q2s[:32])
            q3s = qproj_steps(3, [QP])
            o0s = oproj_steps(0, [OP])
            attention(1, q2s[32:] + o0s)
            o1s = oproj_steps(1, [OP])
            attention(2, q3s + o1s[:32])
            o2s = oproj_steps(2, [OP])
            attention(3, o1s[32:] + o2s)
            # tail: final quarter's output projection (qpool free by now)
            for s in oproj_steps(3, [OP, QP]):
                s()

    nc.finalize()
    return nc


def _host_inputs(q, k, v, Wq, Wk, Wv, Wo):
    """Build the 8 per-core input dicts."""
    inv_freq = 1.0 / (THETA ** (np.arange(0, HD, 2, dtype=np.float32) / HD))
    t = np.arange(S, dtype=np.float32)
    freqs = np.einsum("i,j->ij", t, inv_freq)
    emb = np.concatenate([freqs, freqs], axis=-1)  # [S, 64]
    cosT = np.ascontiguousarray(np.cos(emb).T, dtype=np.float32)  # [64, S]
    sinT = np.ascontiguousarray(np.sin(emb).T, dtype=np.float32)
    cos_rep = np.concatenate([cosT, cosT], axis=0).astype(BF16)  # [128, S]
    sin_rep = np.concatenate([sinT, sinT], axis=0).astype(BF16)

    ident_np = np.eye(P, dtype=np.float32).astype(BF16)
    qT = [np.ascontiguousarray(q[b].T).astype(BF16) for b in range(B)]
    kTt = [np.ascontiguousarray(k[b].T).astype(BF16) for b in range(B)]
    vTt = [np.ascontiguousarray(v[b].T).astype(BF16) for b in range(B)]

    in_maps = []
    for c in range(NCORES):
        b, g = divmod(c, 4)
        # pair order: (kv0-head j, kv1-head j) interleaved
        qheads = [2 * g, 2 * g + 1, 2 * g + 8, 2 * g + 9,
                  2 * g + 16, 2 * g + 17, 2 * g + 24, 2 * g + 25]
        qcols = np.concatenate([np.arange(h * HD, (h + 1) * HD) for h in qheads])
        kvcols = np.arange(2 * g * HD, (2 * g + 2) * HD)

        wq_np = np.ascontiguousarray(Wq[:, qcols]).astype(BF16)
        wk_np = np.ascontiguousarray(Wk[:, kvcols]).astype(BF16)
        wv_np = np.ascontiguousarray(Wv[:, kvcols]).astype(BF16)
        wo_np = np.ascontiguousarray(Wo[qcols, :]).astype(BF16)

        in_maps.append({
            "qT": qT[b], "kT": kTt[b], "vT": vTt[b],
            "wq": wq_np, "wk": wk_np, "wv": wv_np, "wo": wo_np,
            "cosr": cos_rep, "sinr": sin_rep, "ident": ident_np,
        })
    return in_maps


def kernel(q, k, v, attn_mask, Wq, Wk, Wv, Wo, bo):
    from concourse.bass_utils import run_bass_kernel_spmd

    q = np.asarray(q, dtype=np.float32)
    k = np.asarray(k, dtype=np.float32)
    v = np.asarray(v, dtype=np.float32)
    Wq = np.asarray(Wq, dtype=np.float32)
    Wk = np.asarray(Wk, dtype=np.float32)
    Wv = np.asarray(Wv, dtype=np.float32)
    Wo = np.asarray(Wo, dtype=np.float32)
    bo = np.asarray(bo, dtype=np.float32)

    if "nc" not in _CACHE:
        _CACHE["nc"] = _build_program()
    nc = _CACHE["nc"]

    in_maps = _host_inputs(q, k, v, Wq, Wk, Wv, Wo)
    trace = bool(int(os.environ.get("KERNEL_TRACE", "0")))
    tmpdir = os.environ.get("KERNEL_TRACE_DIR") or None
    res = run_bass_kernel_spmd(nc, in_maps, core_ids=list(range(NCORES)),
                               trace=trace, tmpdir=tmpdir)
    _CACHE["last_result"] = res

    out = np.zeros((B, S, D), dtype=np.float32)
    for c in range(NCORES):
        b = c // 4
        out[b] += np.asarray(res.results[c]["out"], dtype=np.float32)
    out += bo[None, None, :]
    return out


# revision 33
# speedup vs baseline: 1.0682x; 1.0682x over previous
"""GQA attention block on 8 trn2 NeuronCores.

Sharding: core c = (batch b=c//4, kv-head-pair g=c%4). Each core owns kv heads
{2g, 2g+1} and their 8 query heads (GQA tile mapping: q-head i -> kv-head i%8),
with Wq/Wk/Wv column-sharded and Wo row-sharded; host sums the 4 partial
outputs per batch (bf16 partials, fp32 sum) and adds bo.

Device strategy (per core):
  - host stages q^T/k^T/v^T (bf16) so every matmul has its contraction dim on
    partitions with no device-side transposes.
  - RoPE applied on DVE (partition-shifted rotate_half + cos/sin combine) for
    both Q and K; no doubled projection weights.
  - q heads are interleaved as (kv0-head j, kv1-head j) pairs so each score
    matmul pair runs ROW-TILED on the PE array (64x128 tiles T0/T8, concurrent)
    writing a 2-bank PSUM blob; one Exp ACT over the [128,1024] blob (scale=1/8
    folded, no max subtraction -- scores bounded ~|6|).
  - AV via lhsT=Vp with an appended ones column giving the softmax denominator
    for free; normalization via fast-approx reciprocal + partition broadcast.
  - out^T feeds the final projection as lhsT directly; partial [S,D] bf16 out.
"""

import os
from contextlib import ExitStack

import numpy as np
import ml_dtypes

D = 2048
QH = 32
KVH = 8
HD = 64
B = 2
S = 2048
THETA = 1000000.0
P = 128
NCORES = 8

BF16 = ml_dtypes.bfloat16

_CACHE = {}


def _build_program():
    import concourse.bass as bass
    import concourse.tile as tile
    from concourse import bacc, mybir

    nc = bacc.Bacc(
        "TRN2",
        target_bir_lowering=False,
        debug=False,
        enable_asserts=False,
        num_devices=NCORES,
    )
    bf = mybir.dt.bfloat16
    f32 = mybir.dt.float32
    Exp = mybir.ActivationFunctionType.Exp
    scale = 1.0 / float(np.sqrt(HD))

    qT = nc.dram_tensor("qT", [D, S], bf, kind="ExternalInput").ap()
    kT = nc.dram_tensor("kT", [D, S], bf, kind="ExternalInput").ap()
    vT = nc.dram_tensor("vT", [D, S], bf, kind="ExternalInput").ap()
    wq = nc.dram_tensor("wq", [D, 512], bf, kind="ExternalInput").ap()
    wk = nc.dram_tensor("wk", [D, 128], bf, kind="ExternalInput").ap()
    wv = nc.dram_tensor("wv", [D, 128], bf, kind="ExternalInput").ap()
    wo = nc.dram_tensor("wo", [512, D], bf, kind="ExternalInput").ap()
    cosr = nc.dram_tensor("cosr", [P, S], f32, kind="ExternalInput").ap()
    sinr = nc.dram_tensor("sinr", [P, S], f32, kind="ExternalInput").ap()
    out = nc.dram_tensor("out", [S, D], bf, kind="ExternalOutput").ap()

    # partitioned DRAM views
    qT3 = qT.rearrange("(o p) s -> p o s", p=P)    # [128, 16, 2048]
    kT3 = kT.rearrange("(o p) s -> p o s", p=P)
    vT3 = vT.rearrange("(o p) s -> p o s", p=P)
    wq3 = wq.rearrange("(o p) m -> p o m", p=P)    # [128, 16, 512]
    wk3 = wk.rearrange("(o p) m -> p o m", p=P)    # [128, 16, 128]
    wv3 = wv.rearrange("(o p) m -> p o m", p=P)    # [128, 16, 128]
    wo3 = wo.rearrange("(o p) d -> p o d", p=P)    # [128, 4, 2048]
    out3 = out.rearrange("(t p) d -> p t d", p=P)  # [128, 16, 2048]

    with tile.TileContext(nc) as tc, ExitStack() as ctx:
        const = ctx.enter_context(tc.tile_pool(name="const", bufs=1))
        persist = ctx.enter_context(tc.tile_pool(name="persist", bufs=1))

        # ---- resident weights / tables (DMA order = need order: wv/wk feed
        # the V/K streams immediately; wo is deferred until after startup) ----
        wv_sb = const.tile([P, 16, 128], bf, tag="wv")
        nc.sync.dma_start(wv_sb[:], wv3[:])
        wk_sb = const.tile([P, 16, 128], bf, tag="wk")
        nc.sync.dma_start(wk_sb[:], wk3[:])
        cos_sb = const.tile([P, S], f32, tag="cos")
        nc.sync.dma_start(cos_sb[:], cosr[:])
        sin_sb = const.tile([P, S], f32, tag="sin")
        nc.sync.dma_start(sin_sb[:], sinr[:])
        wq_sb = const.tile([P, 16, 512], bf, tag="wq")
        nc.sync.dma_start(wq_sb[:], wq3[:])
        wo_sb = const.tile([P, 4, 2048], bf, tag="wo")

        # ---- persistent intermediates ----
        kpt_b = persist.tile([P, S], bf, tag="kpt")          # rotated K^T, kv0|kv1
        qpt_b = persist.tile([P, 4, S], bf, tag="qpt")       # rotated Q^T pairs
        vp_sb = persist.tile([P, 16, 130], bf, tag="vp")     # Vp + ones cols
        outT_b = persist.tile([P, 4, S], bf, tag="outT")     # normalized out^T
        nc.vector.memset(vp_sb[:, :, 64:65], 1.0)
        nc.vector.memset(vp_sb[:, :, 129:130], 1.0)

        def rope_combine(dst, ps, rot_pool, sl):
            """dst[128, 512] (bf16) = ps*cos + rotate_half(ps)*sin over slice sl."""
            rot = rot_pool.tile([P, 512], f32, tag="rot")
            for hh in range(2):
                b0 = hh * 64
                nc.vector.tensor_scalar_mul(
                    rot[b0 : b0 + 32, :], ps[b0 + 32 : b0 + 64, :], -1.0
                )
                nc.vector.tensor_copy(
                    out=rot[b0 + 32 : b0 + 64, :], in_=ps[b0 : b0 + 32, :]
                )
            t1 = rot_pool.tile([P, 512], f32, tag="t1")
            t2 = rot_pool.tile([P, 512], f32, tag="t2")
            nc.vector.tensor_mul(out=t1[:], in0=ps[:], in1=cos_sb[:, sl])
            nc.vector.tensor_mul(out=t2[:], in0=rot[:], in1=sin_sb[:, sl])
            nc.vector.tensor_add(out=dst, in0=t1[:], in1=t2[:])

        # ======= per s-quarter: attention with interleaved Q/O proj =======
        with ExitStack() as mctx:
            bigin = mctx.enter_context(tc.tile_pool(name="bigin2", bufs=2))
            ptmp = mctx.enter_context(tc.tile_pool(name="ptmp2", bufs=1))
            qpool = mctx.enter_context(
                tc.tile_pool(name="qpool", bufs=1, space="PSUM")
            )
            opool = mctx.enter_context(
                tc.tile_pool(name="opool", bufs=1, space="PSUM")
            )
            epool = mctx.enter_context(tc.tile_pool(name="et", bufs=3))
            ntmp = mctx.enter_context(tc.tile_pool(name="ntmp", bufs=2))
            fout = mctx.enter_context(tc.tile_pool(name="fout", bufs=3))

            def qproj_dma(quarter):
                qh_sb = bigin.tile([P, 16, 512], bf, tag="bigin")
                for o in range(16):
                    nc.sync.dma_start(
                        qh_sb[:, o, :], qT3[:, o, quarter * 512 : (quarter + 1) * 512]
                    )
                return qh_sb

            def qproj_steps(quarter, qh_sb, pools):
                """64 tensor-step closures; each emits one matmul (rope attached
                to the last o of each m)."""
                gs = slice(quarter * 512, (quarter + 1) * 512)
                state = {}

                def step(m, o):
                    if o == 0:
                        pool, tag = pools[m % len(pools)]
                        state["ps"] = pool.tile(
                            [P, 512], f32, tag=tag, name=f"qp{quarter}_{m}"
                        )
                    nc.tensor.matmul(
                        state["ps"],
                        lhsT=wq_sb[:, o, m * 128 : (m + 1) * 128],
                        rhs=qh_sb[:, o, :],
                        start=(o == 0),
                        stop=(o == 15),
                    )
                    if o == 15:
                        rope_combine(qpt_b[:, m, gs], state["ps"], ptmp, gs)

                return [
                    (lambda m=m, o=o: step(m, o))
                    for m in range(4)
                    for o in range(16)
                ]

            def oproj_steps(quarter, pools):
                """64 tensor-step closures; copy+DMA attached to cc==3."""
                state = {}

                def step(i, qt, dn, cc):
                    if cc == 0:
                        pool, tag = pools[i % len(pools)]
                        state["psf"] = pool.tile(
                            [P, 512], f32, tag=tag, name=f"psf{quarter}_{i}"
                        )
                    nc.tensor.matmul(
                        state["psf"],
                        lhsT=outT_b[:, cc, qt * 128 : (qt + 1) * 128],
                        rhs=wo_sb[:, cc, dn * 512 : (dn + 1) * 512],
                        start=(cc == 0),
                        stop=(cc == 3),
                    )
                    if cc == 3:
                        of = fout.tile([P, 512], bf, tag="of")
                        nc.vector.tensor_copy(out=of[:], in_=state["psf"][:])
                        nc.sync.dma_start(
                            out3[:, qt, dn * 512 : (dn + 1) * 512], of[:]
                        )

                return [
                    (
                        lambda i=qi * 4 + dn, qt=quarter * 4 + qi, dn=dn, cc=cc:
                        step(i, qt, dn, cc)
                    )
                    for qi in range(4)
                    for dn in range(4)
                    for cc in range(4)
                ]

            def attention(quarter, feeds):
                gs = slice(quarter * 512, (quarter + 1) * 512)
                fed = 0
                nslots = 64
                def emit_av(avA, avB, et, c):
                    nc.tensor.matmul(
                        avA[:],
                        lhsT=vp_sb[:, c, 0:65],
                        rhs=et[:, 0:512],
                        start=(c == 0),
                        stop=(c == 15),
                    )
                    nc.tensor.matmul(
                        avB[:],
                        lhsT=vp_sb[:, c, 65:130],
                        rhs=et[:, 512:1024],
                        start=(c == 0),
                        stop=(c == 15),
                    )

                for j in range(4):
                    avA = avpsum.tile([65, 512], f32, tag="avA")
                    avB = avpsum.tile([65, 512], f32, tag="avB")
                    pend = None  # (et, c) whose AV is deferred one slot
                    for c in range(16):
                        cs = slice(c * 128, (c + 1) * 128)
                        sb = spsum.tile([P, 1024], f32, tag="sb")
                        nc.tensor.matmul(
                            sb[:, 0:512],
                            lhsT=kpt_b[0:64, cs],
                            rhs=qpt_b[0:64, j, gs],
                            start=True,
                            stop=True,
                            tile_position=(0, 0),
                        )
                        nc.tensor.matmul(
                            sb[:, 512:1024],
                            lhsT=kpt_b[64:128, cs],
                            rhs=qpt_b[64:128, j, gs],
                            start=True,
                            stop=True,
                            tile_position=(64, 0),
                        )
                        et = epool.tile([P, 1024], bf, tag="et", name=f"et{c}")
                        nc.scalar.activation(
                            out=et[:], in_=sb[:], func=Exp, scale=scale
                        )
                        # feeds run while this chunk's exp is on the ACT engine
                        slot = j * 16 + c
                        want = (slot + 1) * len(feeds) // nslots
                        while fed < want:
                            feeds[fed]()
                            fed += 1
                        if pend is not None:
                            emit_av(avA, avB, *pend)
                        pend = (et, c)
                    emit_av(avA, avB, *pend)
                    # stage AV out of PSUM immediately (frees the banks for the
                    # next pair); normalize from SBUF afterwards. den rows land
                    # at partition 0 (required by the custom-DVE reciprocal).
                    sdimA = ntmp.tile([64, 512], bf, tag="sdimA")
                    denA = ntmp.tile([1, 512], f32, tag="denA")
                    sdimB = ntmp.tile([64, 512], bf, tag="sdimB")
                    denB = ntmp.tile([1, 512], f32, tag="denB")
                    nc.vector.tensor_copy(out=sdimA[:], in_=avA[0:64, :])
                    nc.vector.tensor_copy(out=denA[:], in_=avA[64:65, :])
                    nc.vector.tensor_copy(out=sdimB[:], in_=avB[0:64, :])
                    nc.vector.tensor_copy(out=denB[:], in_=avB[64:65, :])
                    recA = ntmp.tile([1, 512], f32, tag="recA")
                    recB = ntmp.tile([1, 512], f32, tag="recB")
                    nc.vector.reciprocal_approx_fast(out=recA[:], in_=denA[:])
                    nc.vector.reciprocal_approx_fast(out=recB[:], in_=denB[:])
                    bcA = ntmp.tile([64, 512], f32, tag="bcA")
                    bcB = ntmp.tile([64, 512], f32, tag="bcB")
                    nc.gpsimd.partition_broadcast(bcA[:], recA[:])
                    nc.gpsimd.partition_broadcast(bcB[:], recB[:])
                    nc.vector.tensor_mul(
                        out=outT_b[0:64, j, gs], in0=sdimA[:], in1=bcA[:]
                    )
                    nc.vector.tensor_mul(
                        out=outT_b[64:128, j, gs], in0=sdimB[:], in1=bcB[:]
                    )
                while fed < len(feeds):
                    feeds[fed]()
                    fed += 1

            def interleave(a, b):
                out = []
                for x, y in zip(a, b):
                    out.append(x)
                    out.append(y)
                out += a[len(b):] or b[len(a):]
                return out

            QP = (qpool, "qp")
            OP = (opool, "psf")

            # ===== startup: V + K projections with qproj(0) interleaved =====
            qh0 = qproj_dma(0)
            q0_steps = qproj_steps(0, qh0, [QP, OP])
            q0_fed = 0
            with ExitStack() as pctx:
                kstream = pctx.enter_context(tc.tile_pool(name="kstream", bufs=8))
                vpsum = pctx.enter_context(
                    tc.tile_pool(name="vpsum", bufs=4, space="PSUM")
                )
                kpsum = pctx.enter_context(
                    tc.tile_pool(name="kpsum", bufs=2, space="PSUM")
                )

                for quarter in range(4):
                    # ---- V projection for this quarter: stream per-o tiles,
                    # one PSUM bank per s-tile accumulator ----
                    psv4 = [
                        vpsum.tile([P, 512], f32, tag="ppv", name=f"psv{st}")
                        for st in range(4)
                    ]
                    for o in range(16):
                        vtile = kstream.tile([P, 512], bf, tag="kt", name=f"vt{o}")
                        nc.sync.dma_start(
                            vtile[:], vT3[:, o, quarter * 512 : (quarter + 1) * 512]
                        )
                        for st in range(4):
                            nc.tensor.matmul(
                                psv4[st][:, 0:128],
                                lhsT=vtile[:, st * 128 : (st + 1) * 128],
                                rhs=wv_sb[:, o, :],
                                start=(o == 0),
                                stop=(o == 15),
                            )
                        if o % 4 == 3:
                            while q0_fed < (quarter * 16 + o + 1) * 64 // 80:
                                q0_steps[q0_fed]()
                                q0_fed += 1
                    for st in range(4):
                        kt_idx = quarter * 4 + st
                        nc.vector.tensor_copy(
                            out=vp_sb[:, kt_idx, 0:64], in_=psv4[st][:, 0:64]
                        )
                        nc.vector.tensor_copy(
                            out=vp_sb[:, kt_idx, 65:129], in_=psv4[st][:, 64:128]
                        )

                    # ---- K projection + RoPE for this quarter ----
                    ps_k = kpsum.tile([P, 512], f32, tag="ppk")
                    for o in range(16):
                        ktile = kstream.tile([P, 512], bf, tag="kt")
                        nc.sync.dma_start(
                            ktile[:], kT3[:, o, quarter * 512 : (quarter + 1) * 512]
                        )
                        nc.tensor.matmul(
                            ps_k,
                            lhsT=wk_sb[:, o, :],
                            rhs=ktile[:],
                            start=(o == 0),
                            stop=(o == 15),
                        )
                    sl = slice(quarter * 512, (quarter + 1) * 512)
                    rope_combine(kpt_b[:, sl], ps_k, ptmp, sl)
            while q0_fed < len(q0_steps):
                q0_steps[q0_fed]()
                q0_fed += 1

            # attention-phase PSUM pools (created after startup pools release)
            spsum = mctx.enter_context(
                tc.tile_pool(name="spsum", bufs=2, space="PSUM")
            )
            avpsum = mctx.enter_context(
                tc.tile_pool(name="avpsum", bufs=1, space="PSUM")
            )
            nc.sync.dma_start(wo_sb[:], wo3[:])

            # Balanced feeds: 96 proj steps per quarter (1.5/slot) -- Q
            # projections split across quarter boundaries so the tensor
            # engine stays just under the exp-ACT pace everywhere.
            qh1 = qproj_dma(1)
            q1s = qproj_steps(1, qh1, [QP])
            qh2 = qproj_dma(2)
            q2s = qproj_steps(2, qh2, [QP])
            attention(0, q1s + q2s[:32])
            qh3 = qproj_dma(3)
            q3s = qproj_steps(3, qh3, [QP])
            o0s = oproj_steps(0, [OP])
            attention(1, q2s[32:] + o0s)
            o1s = oproj_steps(1, [OP])
            attention(2, q3s + o1s[:32])
            o2s = oproj_steps(2, [OP])
            attention(3, o1s[32:] + o2s)
            # tail: final quarter's output projection (qpool free by now)
            for s in oproj_steps(3, [OP, QP]):
                s()

    nc.finalize()
    return nc


def _host_inputs(q, k, v, Wq, Wk, Wv, Wo):
    """Build the 8 per-core input dicts."""
    inv_freq = 1.0 / (THETA ** (np.arange(0, HD, 2, dtype=np.float32) / HD))
    t = np.arange(S, dtype=np.float32)
    freqs = np.einsum("i,j->ij", t, inv_freq)
    emb = np.concatenate([freqs, freqs], axis=-1)  # [S, 64]
    cosT = np.ascontiguousarray(np.cos(emb).T, dtype=np.float32)  # [64, S]
    sinT = np.ascontiguousarray(np.sin(emb).T, dtype=np.float32)
    cos_rep = np.concatenate([cosT, cosT], axis=0)  # [128, S]
    sin_rep = np.concatenate([sinT, sinT], axis=0)

    qT = [np.ascontiguousarray(q[b].T).astype(BF16) for b in range(B)]
    kTt = [np.ascontiguousarray(k[b].T).astype(BF16) for b in range(B)]
    vTt = [np.ascontiguousarray(v[b].T).astype(BF16) for b in range(B)]

    in_maps = []
    for c in range(NCORES):
        b, g = divmod(c, 4)
        # pair order: (kv0-head j, kv1-head j) interleaved
        qheads = [2 * g, 2 * g + 1, 2 * g + 8, 2 * g + 9,
                  2 * g + 16, 2 * g + 17, 2 * g + 24, 2 * g + 25]
        qcols = np.concatenate([np.arange(h * HD, (h + 1) * HD) for h in qheads])
        kvcols = np.arange(2 * g * HD, (2 * g + 2) * HD)

        wq_np = np.ascontiguousarray(Wq[:, qcols]).astype(BF16)
        wk_np = np.ascontiguousarray(Wk[:, kvcols]).astype(BF16)
        wv_np = np.ascontiguousarray(Wv[:, kvcols]).astype(BF16)
        wo_np = np.ascontiguousarray(Wo[qcols, :]).astype(BF16)

        in_maps.append({
            "qT": qT[b], "kT": kTt[b], "vT": vTt[b],
            "wq": wq_np, "wk": wk_np, "wv": wv_np, "wo": wo_np,
            "cosr": cos_rep, "sinr": sin_rep,
        })
    return in_maps


def kernel(q, k, v, attn_mask, Wq, Wk, Wv, Wo, bo):
    from concourse.bass_utils import run_bass_kernel_spmd

    q = np.asarray(q, dtype=np.float32)
    k = np.asarray(k, dtype=np.float32)
    v = np.asarray(v, dtype=np.float32)
    Wq = np.asarray(Wq, dtype=np.float32)
    Wk = np.asarray(Wk, dtype=np.float32)
    Wv = np.asarray(Wv, dtype=np.float32)
    Wo = np.asarray(Wo, dtype=np.float32)
    bo = np.asarray(bo, dtype=np.float32)

    if "nc" not in _CACHE:
        _CACHE["nc"] = _build_program()
    nc = _CACHE["nc"]

    in_maps = _host_inputs(q, k, v, Wq, Wk, Wv, Wo)
    trace = bool(int(os.environ.get("KERNEL_TRACE", "0")))
    tmpdir = os.environ.get("KERNEL_TRACE_DIR") or None
    res = run_bass_kernel_spmd(nc, in_maps, core_ids=list(range(NCORES)),
                               trace=trace, tmpdir=tmpdir)
    _CACHE["last_result"] = res

    out = np.zeros((B, S, D), dtype=np.float32)
    for c in range(NCORES):
        b = c // 4
        out[b] += np.asarray(res.results[c]["out"], dtype=np.float32)
    out += bo[None, None, :]
    return out
